# revision 15
# baseline (speedup 1.0000x reference)
"""Multi-head attention (B=4, S=2048, H=16, d_model=1024, d_k=d_v=64) on 8
Trainium2 NeuronCores.

Sharding (v2): 8 cores = 4 batches x 2 head-halves (tensor-parallel over
heads, per the W_Q/W_K/W_V head-split + W_O row-split scheme). Each core
computes 8 heads (4 pairs) over the FULL query range S=2048 for its batch,
projects K/V only for its own heads (no duplicated projection work), runs
its partial output projection against its W_O row block, and the host sums
the two partial outputs per batch (the all-reduce).

Host prep: Q/K/V transposed to [d_model, seq] (V blocked by s-chunk) and
cast to bf16, as are all weights; W_Q/W_K pair-major; W_V/W_O sliced per
head-half.

Per-core pipeline: 8 attention units (pair p, query-half g), pair-major
order. Per unit, the baseline-style software pipeline: scoresT chunk =
kt.T @ qt into PSUM, e = exp(s/8) on ACT (the pacing engine), flipped
value matmul o[q, 65] = e-chunk.T @ [1|v] accumulated over s-chunks with
column 0 the softmax denominator. K/Q/V projection groups and the
output-projection partials run as PE filler inside the ACT-paced loop.

Normalized heads are written qc-major into the unit's dead qt half, then a
single DMA xbar transpose per unit produces pair-stacked headsT in the dead
ktx staging area (no PE transposes). Output projection: partial chains over
pairs 0-2 are staged to SBUF f32 (dead qtx space) as filler; pair-3 tail
matmul + DVE/GPSIMD add completes each chunk (g0 chunks inside unit 7,
g1 chunks in the drain tail).

PSUM note: start_tensor_calc marks the whole 2KB bank pending-zero, so
interleaved per-slot accumulation groups sharing a bank must issue exactly
one start (first slot); the other slots' first writes land on pending-zero
bytes, which the hardware treats as overwrite.
"""

import contextlib
import os
import sys

for _p in ("/opt/trn_rl_repo", "/root/.axon_site/_ro/trn_rl_repo"):
    if os.path.isdir(_p) and _p not in sys.path:
        sys.path.insert(0, _p)

import numpy as np
import ml_dtypes

import concourse.bass as bass  # noqa: F401
import concourse.tile as tile
from concourse import bacc, mybir
from concourse.bass_utils import run_bass_kernel_spmd
from concourse.masks import make_identity

F32 = mybir.dt.float32
F32R = mybir.dt.float32r
BF16 = mybir.dt.bfloat16

B, S, DM = 4, 2048, 1024
H, D = 16, 64
N_CORES = 8
NPC = 4  # head pairs per core (8 heads)
N_SC = S // 128  # kv 128-chunks
N_MO = DM // 128  # model-dim 128-chunks
N_G = 2  # query halves per core
QW = 1024  # query width per attention unit


def build(n_cores=N_CORES, phases=(1, 2, 3), dbg=False):
    nc = bacc.Bacc("TRN2", target_bir_lowering=False, debug=False, num_devices=n_cores)

    # host-transposed activations, bf16 (full batch; core's own head slice
    # of the weights)
    qt_d = nc.dram_tensor("QT", [128, N_MO, S], BF16, kind="ExternalInput").ap()
    kt_d = nc.dram_tensor("KT", [128, N_MO, S], BF16, kind="ExternalInput").ap()
    vt_d = nc.dram_tensor(
        "VTs", [128, N_SC, N_MO, 128], BF16, kind="ExternalInput"
    ).ap()
    # pair-major: [pair, mi=128, mo=8, 128]
    w_q = nc.dram_tensor("WQP", [NPC, 128, N_MO, 128], BF16, kind="ExternalInput").ap()
    w_k = nc.dram_tensor("WKP", [NPC, 128, N_MO, 128], BF16, kind="ExternalInput").ap()
    w_v = nc.dram_tensor("WV3", [128, N_MO, 8 * D], BF16, kind="ExternalInput").ap()
    # [mi=128, pair=4, dm=1024]
    w_o = nc.dram_tensor("WO3", [128, NPC, DM], BF16, kind="ExternalInput").ap()
    out = nc.dram_tensor("out", [S, DM], BF16, kind="ExternalOutput").ap()
    if dbg:
        d_kt = nc.dram_tensor("d_kt", [128, S], BF16, kind="ExternalOutput").ap()
        d_qt = nc.dram_tensor("d_qt", [128, S], BF16, kind="ExternalOutput").ap()
        d_vall = nc.dram_tensor(
            "d_vall", [128, N_SC * 8 * 65], BF16, kind="ExternalOutput"
        ).ap()
        d_onorm = nc.dram_tensor("d_onorm", [128, QW], BF16, kind="ExternalOutput").ap()
        d_heads = nc.dram_tensor("d_heads", [128, S], BF16, kind="ExternalOutput").ap()

    with tile.TileContext(nc) as tc:
        with (
            tc.tile_pool(name="pers", bufs=1) as pers,
            tc.tile_pool(name="wkq", bufs=2) as wkq,
            tc.tile_pool(name="wop", bufs=1) as wop,
        ):
            wo_sb = wop.tile([128, NPC, DM], BF16, tag="wo", name="wo")
            ident_f32 = wop.tile([128, 128], F32, tag="identf", name="ident_f32")
            make_identity(nc, ident_f32[:])
            ident_bf = wop.tile([128, 128], BF16, tag="ident", name="ident_bf")
            nc.vector.tensor_copy(ident_bf[:], ident_f32[:])
            # v resident: per s-chunk block of 8 head-slots [1|v] (65 wide)
            v_all = pers.tile([128, N_SC, 8, 65], BF16, tag="v_all")
            nc.vector.memset(v_all[:, :, :, 0:1], 1.0)
            # kt[p]: pair-stacked [2*64, S]; qt[p]: [128, S], whose g-halves
            # are later reused for normalized flipped heads (qc-major)
            kt_sb = [
                pers.tile([128, S], BF16, tag=f"kt{p}", name=f"kt{p}")
                for p in range(NPC)
            ]
            qt_sb = [
                pers.tile([128, S], BF16, tag=f"qt{p}", name=f"qt{p}")
                for p in range(NPC)
            ]
            # resident staged transposed inputs for projections; ktx rows
            # double as headsT homes (mo-slot p holds pair p's headsT) once
            # the K projections have consumed them
            ktx = pers.tile([128, N_MO, S], BF16, tag="ktx")
            qtx = pers.tile([128, N_MO * S], BF16, tag="qtx")

            def heads_home(p):
                # pairs 0-1: own kt tile (dead after their second unit);
                # pairs 2-3: ktx slots (K staging is fully consumed by the
                # time their transposes run)
                return kt_sb[p] if p < 2 else ktx[:, p, :]

            def heads_dst(p, g):
                # [128, 8, 128] chunked-transpose target
                return heads_home(p)[:, g * QW : (g + 1) * QW].rearrange(
                    "p (c q) -> p c q", q=128
                )

            def heads_chunk(p, qc):
                # out-projection stationary: [128 hv, 128 q] for global qc
                return heads_home(p)[:, qc * 128 : (qc + 1) * 128]

            def stage_slot(i):
                # 16 bf16 staging slots in dead qtx space (bf16 so the tail
                # identity-matmul re-feed stays a plain bf16 matmul)
                return qtx[:, i * 512 : (i + 1) * 512]

            with (
                tc.tile_pool(name="psum_sp", bufs=1, space="PSUM") as spsum,
                tc.tile_pool(name="psum_o", bufs=1, space="PSUM") as opsum,
                tc.tile_pool(name="psum_pj", bufs=1, space="PSUM") as pjsum,
                tc.tile_pool(name="epool", bufs=4) as epool,
                tc.tile_pool(name="npool", bufs=1) as npool,
                tc.tile_pool(name="fout", bufs=2) as fout,
            ):
                ves = contextlib.ExitStack()
                vpool = ves.enter_context(tc.tile_pool(name="vpool", bufs=1))
                vtx = vpool.tile([128, N_SC, N_MO, 128], BF16, tag="vtx")
                wv_sb = vpool.tile([128, N_MO, 8 * D], BF16, tag="wv")

                cur_wk, cur_wq = {}, {}

                def prefetch_wk(p):
                    wkt = wkq.tile([128, N_MO, 128], BF16, tag="wk", name="wk")
                    cur_wk[p] = wkt
                    nc.sync.dma_start(out=wkt[:], in_=w_k[p])

                def prefetch_wq(p):
                    wqt = wkq.tile([128, N_MO, 128], BF16, tag="wq", name="wq")
                    cur_wq[p] = wqt
                    nc.sync.dma_start(out=wqt[:], in_=w_q[p])

                def dma_ktx(g):
                    nc.sync.dma_start(
                        out=ktx[:, :, g * 512 : (g + 1) * 512],
                        in_=kt_d[:, :, g * 512 : (g + 1) * 512],
                    )

                def dma_qtx(g):
                    nc.sync.dma_start(
                        out=qtx[:].rearrange("p (m s) -> p m s", s=S)[
                            :, :, g * 512 : (g + 1) * 512
                        ],
                        in_=qt_d[:, :, g * 512 : (g + 1) * 512],
                    )

                def dma_wv(first):
                    if first:
                        nc.sync.dma_start(
                            out=wv_sb[:, :, 0:256], in_=w_v[:, :, 0:256]
                        )
                    else:
                        nc.sync.dma_start(
                            out=wv_sb[:, :, 256:512], in_=w_v[:, :, 256:512]
                        )

                def dma_vt(blk):
                    nc.sync.dma_start(
                        out=vtx[:, 4 * blk : 4 * blk + 4, :, :],
                        in_=vt_d[:, 4 * blk : 4 * blk + 4, :, :],
                    )

                # startup-critical DMA order: pair-0 K/Q weights + first K/Q
                # columns, then V / remaining K/Q interleaved
                prefetch_wk(0)
                prefetch_wq(0)
                dma_ktx(0)
                dma_qtx(0)
                dma_qtx(1)
                dma_wv(True)
                dma_vt(0)
                prefetch_wk(1)
                prefetch_wq(1)
                dma_ktx(1)
                dma_vt(1)
                dma_ktx(2)
                dma_vt(2)
                dma_ktx(3)
                dma_qtx(2)
                dma_vt(3)
                dma_qtx(3)
                dma_wv(False)

                # ---------- projection "filler" groups ----------
                def k_group(p, g):
                    pj = pjsum.tile([128, 512], F32, tag="pj")
                    for mo in range(N_MO):
                        nc.tensor.matmul(
                            pj[:],
                            cur_wk[p][:, mo, :],
                            ktx[:, mo, g * 512 : (g + 1) * 512],
                            start=(mo == 0),
                            stop=(mo == N_MO - 1),
                        )
                    nc.vector.tensor_copy(kt_sb[p][:, g * 512 : (g + 1) * 512], pj[:])

                def q_group(p, g):
                    pj = pjsum.tile([128, 512], F32, tag="pj")
                    for mo in range(N_MO):
                        nc.tensor.matmul(
                            pj[:],
                            cur_wq[p][:, mo, :],
                            qtx[:, mo * S + g * 512 : mo * S + (g + 1) * 512],
                            start=(mo == 0),
                            stop=(mo == N_MO - 1),
                        )
                    nc.vector.tensor_copy(qt_sb[p][:, g * 512 : (g + 1) * 512], pj[:])

                def v_group(sc, hp):
                    # one pair's two head-slots (128 wv cols), chunk sc
                    pj = pjsum.tile([128, 512], F32, tag="pj")
                    for mo in range(N_MO):
                        nc.tensor.matmul(
                            pj[:, 0:128],
                            vtx[:, sc, mo, :],
                            wv_sb[:, mo, hp * 128 : (hp + 1) * 128],
                            start=(mo == 0),
                            stop=(mo == N_MO - 1),
                        )
                    nc.vector.tensor_copy(
                        v_all[:, sc, 2 * hp : 2 * hp + 2, 1:65],
                        pj[:, 0:128].rearrange("p (h w) -> p h w", h=2),
                    )

                # out-projection:
                #  - g1 chunks: partial chains over pairs 0-2 staged to SBUF
                #    f32 (dead qtx space) inside units 6-7; the drain tail
                #    re-feeds each staged partial into PSUM with an identity
                #    matmul (float32r moving operand: full rate, ~tf32
                #    rounding) on top of the pair-3 matmul, then DMAs the
                #    PSUM result out directly.
                #  - g0 chunks: full 4-pair chains inside unit 7.
                def stage(qc, dmc):
                    pj = pjsum.tile([128, 512], F32, tag="pj", name="st")
                    for p2 in range(NPC - 1):
                        nc.tensor.matmul(
                            pj[:],
                            heads_chunk(p2, qc),
                            wo_sb[:, p2, dmc * 512 : (dmc + 1) * 512],
                            start=(p2 == 0),
                            stop=(p2 == NPC - 2),
                        )
                    nc.vector.tensor_copy(stage_slot((qc - 8) * 2 + dmc), pj[:])

                def full_chain(qc):
                    # both dm-halves of one q-chunk -> one bf16 store
                    fo = fout.tile([128, 1024], BF16, tag="fo")
                    for dmc in range(2):
                        pj = pjsum.tile([128, 512], F32, tag="pj", name="fc")
                        for p2 in range(NPC):
                            nc.tensor.matmul(
                                pj[:],
                                heads_chunk(p2, qc),
                                wo_sb[:, p2, dmc * 512 : (dmc + 1) * 512],
                                start=(p2 == 0),
                                stop=(p2 == NPC - 1),
                            )
                        nc.vector.tensor_copy(
                            fo[:, dmc * 512 : (dmc + 1) * 512], pj[:]
                        )
                    nc.sync.dma_start(out=out[qc * 128 : (qc + 1) * 128, :], in_=fo[:])

                # K/Q for unit (0,0) before attention starts
                k_group(0, 0)
                q_group(0, 0)
                q_group(0, 1)

                if dbg:
                    nc.sync.dma_start(out=d_kt[:], in_=kt_sb[0][:])
                    nc.sync.dma_start(out=d_qt[:], in_=qt_sb[0][:])

                def spread(items, nsteps=N_SC):
                    outl = [[] for _ in range(nsteps)]
                    for i, it in enumerate(items):
                        outl[(i * nsteps) // len(items)].append(it)
                    return outl

                def K(p, g):
                    return lambda: k_group(p, g)

                def Q(p, g):
                    return lambda: q_group(p, g)

                def V(sc, hp):
                    return lambda: v_group(sc, hp)

                def ST(qc, dmc):
                    return lambda: stage(qc, dmc)

                def FC(qc):
                    return lambda: full_chain(qc)

                def PF(p):
                    return [lambda p=p: prefetch_wk(p), lambda p=p: prefetch_wq(p)]

                # per-unit filler schedules, keyed by unit index in
                # U_ORDER = (0,g0),(1,g0),(0,g1),(1,g1),(2,g0),(3,g0),
                # (2,g1),(3,g1). Deadlines: v(sc,hp) by pair hp's first
                # unit, step sc+2; k(p,g) by pair p's first unit, step 4g;
                # q(p,gh) by the start of unit (p,gh). Entries are ordered
                # by staging-DMA arrival so a stalled group never blocks
                # the single projection-PSUM bank for a ready one.
                sched = {}
                sched[0] = [[] for _ in range(N_SC)]
                for step, fills in {
                    0: [V(0, 0), V(1, 0)],
                    1: [V(2, 0)],
                    2: [V(3, 0)],
                    3: [K(0, 1)],
                    4: [V(4, 0)],
                    5: [V(5, 0), K(1, 0)],
                    6: [V(6, 0)],
                    7: [V(7, 0), K(0, 2)],
                    8: [V(8, 0), Q(1, 0)],
                    9: [V(9, 0)],
                    10: [V(10, 0), K(0, 3)],
                    11: [V(11, 0), Q(1, 1)],
                    12: [V(12, 0)],
                    13: [V(13, 0)],
                    14: [V(14, 0), V(0, 1)],
                    15: [V(15, 0), V(1, 1)],
                }.items():
                    sched[0][step] = fills
                sched[1] = [[] for _ in range(N_SC)]
                for step, fills in {
                    0: [V(2, 1)],
                    1: [V(3, 1), K(1, 1)],
                    2: [V(4, 1)],
                    3: [V(5, 1)],
                    4: [V(6, 1), K(1, 2)],
                    5: [V(7, 1)],
                    6: [V(8, 1)],
                    7: [V(9, 1), K(1, 3)],
                    8: [V(10, 1)] + PF(2),
                    9: [V(11, 1)],
                    10: [V(12, 1), Q(0, 2)],
                    11: [V(13, 1)],
                    12: [V(14, 1), Q(0, 3)],
                    13: [V(15, 1)],
                }.items():
                    sched[1][step] = fills
                sched[2] = spread([Q(1, 2), Q(1, 3), K(2, 0), K(2, 1)])
                sched[3] = spread(
                    PF(3)
                    + [V(sc, 2) for sc in range(10)]
                    + [K(2, 2), K(2, 3)]
                    + [Q(2, 0), Q(2, 1)]
                )
                sched[4] = [[] for _ in range(N_SC)]
                for step, fills in {
                    0: [K(3, 0)],
                    1: [V(0, 3)],
                    2: [K(3, 1)],
                    3: [V(1, 3)],
                    4: [K(3, 2)],
                    5: [V(2, 3)],
                    6: [K(3, 3)],
                    7: [Q(3, 0), V(3, 3)],
                    8: [V(10, 2), V(4, 3)],
                    9: [Q(3, 1), V(11, 2)],
                    10: [V(12, 2), V(5, 3)],
                    11: [V(13, 2)],
                    12: [V(14, 2)],
                    13: [V(15, 2)],
                }.items():
                    sched[4][step] = fills
                sched[5] = spread(
                    [V(sc, 3) for sc in range(6, N_SC)] + [Q(2, 2), Q(2, 3)]
                )
                sched[6] = spread(
                    [Q(3, 2), Q(3, 3)] + [FC(qc) for qc in range(8)]
                )
                sched[7] = spread(
                    [ST(qc, dmc) for qc in range(8, 16) for dmc in range(2)]
                )

                # o accumulator slot -> AP. 16 slots (h,qc) packed into PSUM
                # banks as 7+7+2 (bank-straddle constraint).
                def o_slot(tiles, h, qc):
                    s = h * 8 + qc
                    if s < 7:
                        return tiles[0][:, s, :]
                    if s < 14:
                        return tiles[1][:, s - 7, :]
                    return tiles[2][:, s - 14, :]

                def emit_o(p, sc, h, o_ps, e_tiles):
                    e_sb = e_tiles.pop((sc, h))
                    for qc in range(8):
                        s = h * 8 + qc
                        nc.tensor.matmul(
                            o_slot(o_ps, h, qc),
                            e_sb[:, qc * 128 : (qc + 1) * 128],
                            v_all[:, sc, 2 * p + h, :],
                            start=(sc == 0 and s in (0, 7, 14)),
                            stop=(sc == N_SC - 1),
                            skip_group_check=True,
                        )

                # ---------------- attention (phase 2) ----------------
                U_ORDER = [(0, 0), (1, 0), (0, 1), (1, 1), (2, 0), (3, 0), (2, 1), (3, 1)]
                for u in range(2 * NPC if 2 in phases else 0):
                    p, g = U_ORDER[u]
                    if dbg and u == 2:
                        nc.sync.dma_start(out=d_onorm[:], in_=qt_sb[0][:, 0:QW])
                    if dbg and u == 6:
                        nc.sync.dma_start(
                            out=d_vall[:],
                            in_=v_all[:].rearrange("p a b c -> p (a b c)"),
                        )
                    if u == 6:
                        # V work done; release its staging space and load W_O
                        ves.close()
                        nc.sync.dma_start(out=wo_sb[:], in_=w_o[:])
                    o_ps = [
                        opsum.tile([128, 7, 65], F32, tag="oA", name="oA"),
                        opsum.tile([128, 7, 65], F32, tag="oB", name="oB"),
                        opsum.tile([128, 2, 65], F32, tag="oC", name="oC"),
                    ]
                    e_tiles = {}
                    for sc in range(N_SC):
                        for h in range(2):
                            lo, hi = h * 64, h * 64 + 64
                            sp = spsum.tile([128, QW], F32, tag=f"sp{h}")
                            for qc2 in range(QW // 512):
                                nc.tensor.matmul(
                                    sp[:, qc2 * 512 : (qc2 + 1) * 512],
                                    kt_sb[p][lo:hi, sc * 128 : (sc + 1) * 128],
                                    qt_sb[p][
                                        lo:hi,
                                        g * QW + qc2 * 512 : g * QW + (qc2 + 1) * 512,
                                    ],
                                    start=True,
                                    stop=True,
                                    skip_group_check=True,
                                )
                            e_sb = epool.tile([128, QW], BF16, tag=f"e{h}")
                            nc.scalar.activation(
                                e_sb[:],
                                sp[:],
                                mybir.ActivationFunctionType.Exp,
                                scale=0.125,
                            )
                            e_tiles[(sc, h)] = e_sb
                        # value matmuls lag two s-chunks behind the scores
                        if sc > 1:
                            for h in range(2):
                                emit_o(p, sc - 2, h, o_ps, e_tiles)
                        for fill in sched[u][sc]:
                            fill()
                    for h in range(2):
                        emit_o(p, N_SC - 2, h, o_ps, e_tiles)
                    emit_o(p, N_SC - 1, 0, o_ps, e_tiles)
                    # drain PSUM -> SBUF (bf16) + f32 denominators; oA only
                    # holds h=0 slots so it can drain while h=1 value
                    # matmuls still run
                    o_sb = npool.tile([128, 16, 65], BF16, tag="osb")
                    den = npool.tile([128, 16], F32, tag="den")
                    nc.vector.tensor_copy(o_sb[:, 0:7, :], o_ps[0][:])
                    nc.vector.tensor_copy(den[:, 0:7, None], o_ps[0][:, :, 0:1])
                    emit_o(p, N_SC - 1, 1, o_ps, e_tiles)
                    nc.vector.tensor_copy(o_sb[:, 7:14, :], o_ps[1][:])
                    nc.vector.tensor_copy(o_sb[:, 14:16, :], o_ps[2][:])
                    nc.vector.tensor_copy(den[:, 7:14, None], o_ps[1][:, :, 0:1])
                    nc.vector.tensor_copy(den[:, 14:16, None], o_ps[2][:, :, 0:1])
                    rec = npool.tile([128, 16], F32, tag="rec")
                    nc.vector.reciprocal_approx_fast(rec[:], den[:])
                    # normalized flipped heads, qc-major, into the dead qt
                    # half: col = qc*128 + h*64 + v
                    o_norm = qt_sb[p][:, g * QW : (g + 1) * QW].rearrange(
                        "p (a b c) -> p b a c", a=8, b=2, c=64
                    )
                    nc.vector.tensor_mul(
                        o_norm,
                        o_sb[:, :, 1:65].rearrange("p (h q) c -> p h q c", h=2),
                        rec[:].rearrange("p (h q) -> p h q", h=2)[
                            :, :, :, None
                        ].broadcast_to([128, 2, 8, 64]),
                    )
                    # xbar transposes -> pair-stacked headsT, emitted only
                    # once the destination is dead: pairs 0-1 overwrite
                    # their kt tile after their second (g1) unit; pairs 2-3
                    # write ktx right at their own unit's end (all K
                    # projection groups have been emitted by then)
                    if p < 2 and g == 1:
                        for g2 in range(2):
                            nc.sync.dma_start_transpose(
                                out=heads_dst(p, g2),
                                in_=qt_sb[p][:, g2 * QW : (g2 + 1) * QW],
                            )
                    elif p >= 2:
                        nc.sync.dma_start_transpose(
                            out=heads_dst(p, g),
                            in_=qt_sb[p][:, g * QW : (g + 1) * QW],
                        )

                if dbg:
                    nc.sync.dma_start(out=d_heads[:], in_=kt_sb[0][:])

                # drain unused fillers (for phases subsets)
                if 2 not in phases:
                    ves.close()
                    nc.sync.dma_start(out=wo_sb[:], in_=w_o[:])
                    for u in range(2 * NPC):
                        for step in sched.get(u, []):
                            for fill in step:
                                fill()

            # ---------------- g1 out-projection tails ----------------
            # pair-3 matmul plus an identity(float32r) re-feed of the staged
            # pairs-0-2 partial into the same PSUM accumulation, then DMA
            # the finished chunk straight from PSUM.
            with (
                tc.tile_pool(name="psum_t", bufs=3, space="PSUM") as tpsum,
                tc.tile_pool(name="fout2", bufs=4) as fout2,
            ):
                if 3 in phases:
                    def tail2(qc):
                        fo = fout2.tile([128, 1024], BF16, tag="fo")
                        for dmc in range(2):
                            tl = tpsum.tile([128, 512], F32, tag="tl")
                            nc.tensor.matmul(
                                tl[:],
                                heads_chunk(NPC - 1, qc),
                                wo_sb[:, NPC - 1, dmc * 512 : (dmc + 1) * 512],
                                start=True,
                                stop=False,
                            )
                            nc.tensor.matmul(
                                tl[:],
                                ident_bf[:],
                                stage_slot((qc - 8) * 2 + dmc),
                                start=False,
                                stop=True,
                            )
                            if dmc == 0:
                                nc.scalar.copy(fo[:, 0:512], tl[:])
                            else:
                                nc.vector.tensor_copy(fo[:, 512:1024], tl[:])
                        nc.sync.dma_start(
                            out=out[qc * 128 : (qc + 1) * 128, :], in_=fo[:]
                        )

                    for qc in range(8, 16):
                        tail2(qc)
    nc.compile()
    return nc


_NC_CACHE = {}


def _get_nc():
    if "nc" not in _NC_CACHE:
        _NC_CACHE["nc"] = build()
    return _NC_CACHE["nc"]


def _prep_w3p(w):
    # [H, DM, D] -> pair-major [8 pairs, mi=128, mo=8, 128], bf16
    w3 = w.transpose(1, 0, 2).reshape(N_MO, 128, H * D).transpose(1, 0, 2)
    return np.ascontiguousarray(
        w3.reshape(128, N_MO, H // 2, 128).transpose(2, 0, 1, 3)
    ).astype(ml_dtypes.bfloat16)


def _prep_wv(w):
    # [H, DM, D] -> [mi=128, mo=8, (h d)=1024], bf16
    return np.ascontiguousarray(
        w.transpose(1, 0, 2).reshape(N_MO, 128, H * D).transpose(1, 0, 2)
    ).astype(ml_dtypes.bfloat16)


def _prep_wo(w):
    # [H*D=1024, DM] -> [mi=128, chunk=8, DM], bf16
    return np.ascontiguousarray(w.reshape(8, 128, DM).transpose(1, 0, 2)).astype(
        ml_dtypes.bfloat16
    )


def _prep_xt(x):
    # [S, DM] -> transposed [128(dm within mo), mo=8, S], bf16
    return np.ascontiguousarray(
        x.T.reshape(N_MO, 128, x.shape[0]).transpose(1, 0, 2)
    ).astype(ml_dtypes.bfloat16)


def _prep_vt(x):
    # [S, DM] -> [128(dm within mo), sc=16, mo=8, 128(s within chunk)]
    return np.ascontiguousarray(
        x.reshape(N_SC, 128, N_MO, 128).transpose(3, 0, 2, 1)
    ).astype(ml_dtypes.bfloat16)


def kernel(Q, K, V, W_Q, W_K, W_V, W_O, _trace=False):
    Q = np.asarray(Q, dtype=np.float32)
    K = np.asarray(K, dtype=np.float32)
    V = np.asarray(V, dtype=np.float32)
    wq8 = _prep_w3p(np.asarray(W_Q, dtype=np.float32))  # [8 pairs, ...]
    wk8 = _prep_w3p(np.asarray(W_K, dtype=np.float32))
    wv8 = _prep_wv(np.asarray(W_V, dtype=np.float32))  # [128, 8, 1024]
    wo8 = _prep_wo(np.asarray(W_O, dtype=np.float32))  # [128, 8, DM]

    qt_b = [_prep_xt(Q[b]) for b in range(B)]
    kt_b = [_prep_xt(K[b]) for b in range(B)]
    vt_b = [_prep_vt(V[b]) for b in range(B)]
    wq_h = [np.ascontiguousarray(wq8[hh * NPC : (hh + 1) * NPC]) for hh in range(2)]
    wk_h = [np.ascontiguousarray(wk8[hh * NPC : (hh + 1) * NPC]) for hh in range(2)]
    wv_h = [
        np.ascontiguousarray(wv8[:, :, hh * 512 : (hh + 1) * 512]) for hh in range(2)
    ]
    wo_h = [
        np.ascontiguousarray(wo8[:, hh * NPC : (hh + 1) * NPC, :]) for hh in range(2)
    ]

    in_maps = []
    for c in range(N_CORES):
        b, hh = c // 2, c % 2
        in_maps.append(
            {
                "QT": qt_b[b],
                "KT": kt_b[b],
                "VTs": vt_b[b],
                "WQP": wq_h[hh],
                "WKP": wk_h[hh],
                "WV3": wv_h[hh],
                "WO3": wo_h[hh],
            }
        )

    nc = _get_nc()
    res = run_bass_kernel_spmd(nc, in_maps, list(range(N_CORES)), trace=_trace)
    out = np.empty((B, S, DM), dtype=np.float32)
    for b in range(B):
        out[b] = np.asarray(res.results[2 * b]["out"], dtype=np.float32) + np.asarray(
            res.results[2 * b + 1]["out"], dtype=np.float32
        )
    if _trace:
        kernel._last_results = res
    return out


# revision 17
# speedup vs baseline: 1.0125x; 1.0125x over previous
"""Multi-head attention (B=4, S=2048, H=16, d_model=1024, d_k=d_v=64) on 8
Trainium2 NeuronCores.

Sharding (v2): 8 cores = 4 batches x 2 head-halves (tensor-parallel over
heads, per the W_Q/W_K/W_V head-split + W_O row-split scheme). Each core
computes 8 heads (4 pairs) over the FULL query range S=2048 for its batch,
projects K/V only for its own heads (no duplicated projection work), runs
its partial output projection against its W_O row block, and the host sums
the two partial outputs per batch (the all-reduce).

Host prep: Q/K/V transposed to [d_model, seq] (V blocked by s-chunk) and
cast to bf16, as are all weights; W_Q/W_K pair-major; W_V/W_O sliced per
head-half.

Per-core pipeline: 8 attention units (pair p, query-half g), pair-major
order. Per unit, the baseline-style software pipeline: scoresT chunk =
kt.T @ qt into PSUM, e = exp(s/8) on ACT (the pacing engine), flipped
value matmul o[q, 65] = e-chunk.T @ [1|v] accumulated over s-chunks with
column 0 the softmax denominator. K/Q/V projection groups and the
output-projection partials run as PE filler inside the ACT-paced loop.

Normalized heads are written qc-major into the unit's dead qt half, then a
single DMA xbar transpose per unit produces pair-stacked headsT in the dead
ktx staging area (no PE transposes). Output projection: partial chains over
pairs 0-2 are staged to SBUF f32 (dead qtx space) as filler; pair-3 tail
matmul + DVE/GPSIMD add completes each chunk (g0 chunks inside unit 7,
g1 chunks in the drain tail).

PSUM note: start_tensor_calc marks the whole 2KB bank pending-zero, so
interleaved per-slot accumulation groups sharing a bank must issue exactly
one start (first slot); the other slots' first writes land on pending-zero
bytes, which the hardware treats as overwrite.
"""

import contextlib
import os
import sys

for _p in ("/opt/trn_rl_repo", "/root/.axon_site/_ro/trn_rl_repo"):
    if os.path.isdir(_p) and _p not in sys.path:
        sys.path.insert(0, _p)

import numpy as np
import ml_dtypes

import concourse.bass as bass  # noqa: F401
import concourse.tile as tile
from concourse import bacc, mybir
from concourse.bass_utils import run_bass_kernel_spmd
from concourse.masks import make_identity

F32 = mybir.dt.float32
F32R = mybir.dt.float32r
BF16 = mybir.dt.bfloat16

B, S, DM = 4, 2048, 1024
H, D = 16, 64
N_CORES = 8
NPC = 4  # head pairs per core (8 heads)
N_SC = S // 128  # kv 128-chunks
N_MO = DM // 128  # model-dim 128-chunks
N_G = 2  # query halves per core
QW = 1024  # query width per attention unit


def build(n_cores=N_CORES, phases=(1, 2, 3), dbg=False):
    nc = bacc.Bacc("TRN2", target_bir_lowering=False, debug=False, num_devices=n_cores)

    # host-transposed activations, bf16 (full batch; core's own head slice
    # of the weights)
    qt_d = nc.dram_tensor("QT", [128, N_MO, S], BF16, kind="ExternalInput").ap()
    kt_d = nc.dram_tensor("KT", [128, N_MO, S], BF16, kind="ExternalInput").ap()
    vt_d = nc.dram_tensor(
        "VTs", [128, N_SC, N_MO, 128], BF16, kind="ExternalInput"
    ).ap()
    # pair-major: [pair, mi=128, mo=8, 128]
    w_q = nc.dram_tensor("WQP", [NPC, 128, N_MO, 128], BF16, kind="ExternalInput").ap()
    w_k = nc.dram_tensor("WKP", [NPC, 128, N_MO, 128], BF16, kind="ExternalInput").ap()
    w_v = nc.dram_tensor("WV3", [128, N_MO, 8 * D], BF16, kind="ExternalInput").ap()
    # [mi=128, pair=4, dm=1024]
    w_o = nc.dram_tensor("WO3", [128, NPC, DM], BF16, kind="ExternalInput").ap()
    out = nc.dram_tensor("out", [S, DM], BF16, kind="ExternalOutput").ap()
    if dbg:
        d_kt = nc.dram_tensor("d_kt", [128, S], BF16, kind="ExternalOutput").ap()
        d_qt = nc.dram_tensor("d_qt", [128, S], BF16, kind="ExternalOutput").ap()
        d_vall = nc.dram_tensor(
            "d_vall", [128, N_SC * 8 * 65], BF16, kind="ExternalOutput"
        ).ap()
        d_onorm = nc.dram_tensor("d_onorm", [128, QW], BF16, kind="ExternalOutput").ap()
        d_heads = nc.dram_tensor("d_heads", [128, S], BF16, kind="ExternalOutput").ap()

    with tile.TileContext(nc) as tc:
        with (
            tc.tile_pool(name="pers", bufs=1) as pers,
            tc.tile_pool(name="wkq", bufs=2) as wkq,
            tc.tile_pool(name="wop", bufs=1) as wop,
        ):
            wo_sb = wop.tile([128, NPC, DM], BF16, tag="wo", name="wo")
            ident_f32 = wop.tile([128, 128], F32, tag="identf", name="ident_f32")
            make_identity(nc, ident_f32[:])
            ident_bf = wop.tile([128, 128], BF16, tag="ident", name="ident_bf")
            nc.vector.tensor_copy(ident_bf[:], ident_f32[:])
            # v resident: per s-chunk block of 8 head-slots [1|v] (65 wide)
            v_all = pers.tile([128, N_SC, 8, 65], BF16, tag="v_all")
            nc.vector.memset(v_all[:, :, :, 0:1], 1.0)
            # kt[p]: pair-stacked [2*64, S]; qt[p]: [128, S], whose g-halves
            # are later reused for normalized flipped heads (qc-major)
            kt_sb = [
                pers.tile([128, S], BF16, tag=f"kt{p}", name=f"kt{p}")
                for p in range(NPC)
            ]
            qt_sb = [
                pers.tile([128, S], BF16, tag=f"qt{p}", name=f"qt{p}")
                for p in range(NPC)
            ]
            # resident staged transposed inputs for projections; ktx rows
            # double as headsT homes (mo-slot p holds pair p's headsT) once
            # the K projections have consumed them
            ktx = pers.tile([128, N_MO, S], BF16, tag="ktx")
            qtx = pers.tile([128, N_MO * S], BF16, tag="qtx")

            def heads_home(p):
                # pairs 0-1: own kt tile (dead after their second unit);
                # pairs 2-3: ktx slots (K staging is fully consumed by the
                # time their transposes run)
                return kt_sb[p] if p < 2 else ktx[:, p, :]

            def heads_dst(p, g):
                # [128, 8, 128] chunked-transpose target
                return heads_home(p)[:, g * QW : (g + 1) * QW].rearrange(
                    "p (c q) -> p c q", q=128
                )

            def heads_chunk(p, qc):
                # out-projection stationary: [128 hv, 128 q] for global qc
                return heads_home(p)[:, qc * 128 : (qc + 1) * 128]

            def stage_slot(i):
                # 16 bf16 staging slots in dead qtx space (bf16 so the tail
                # identity-matmul re-feed stays a plain bf16 matmul)
                return qtx[:, i * 512 : (i + 1) * 512]

            with (
                tc.tile_pool(name="psum_sp", bufs=1, space="PSUM") as spsum,
                tc.tile_pool(name="psum_o", bufs=1, space="PSUM") as opsum,
                tc.tile_pool(name="psum_pj", bufs=1, space="PSUM") as pjsum,
                tc.tile_pool(name="epool", bufs=4) as epool,
                tc.tile_pool(name="npool", bufs=1) as npool,
                tc.tile_pool(name="fout", bufs=2) as fout,
            ):
                ves = contextlib.ExitStack()
                vpool = ves.enter_context(tc.tile_pool(name="vpool", bufs=1))
                vtx = vpool.tile([128, N_SC, N_MO, 128], BF16, tag="vtx")
                wv_sb = vpool.tile([128, N_MO, 8 * D], BF16, tag="wv")

                cur_wk, cur_wq = {}, {}

                def prefetch_wk(p):
                    wkt = wkq.tile([128, N_MO, 128], BF16, tag="wk", name="wk")
                    cur_wk[p] = wkt
                    nc.sync.dma_start(out=wkt[:], in_=w_k[p])

                def prefetch_wq(p):
                    wqt = wkq.tile([128, N_MO, 128], BF16, tag="wq", name="wq")
                    cur_wq[p] = wqt
                    nc.sync.dma_start(out=wqt[:], in_=w_q[p])

                def dma_ktx(g):
                    nc.sync.dma_start(
                        out=ktx[:, :, g * 512 : (g + 1) * 512],
                        in_=kt_d[:, :, g * 512 : (g + 1) * 512],
                    )

                def dma_qtx(g):
                    nc.sync.dma_start(
                        out=qtx[:].rearrange("p (m s) -> p m s", s=S)[
                            :, :, g * 512 : (g + 1) * 512
                        ],
                        in_=qt_d[:, :, g * 512 : (g + 1) * 512],
                    )

                def dma_wv(first):
                    if first:
                        nc.sync.dma_start(
                            out=wv_sb[:, :, 0:256], in_=w_v[:, :, 0:256]
                        )
                    else:
                        nc.sync.dma_start(
                            out=wv_sb[:, :, 256:512], in_=w_v[:, :, 256:512]
                        )

                def dma_vt(blk):
                    nc.sync.dma_start(
                        out=vtx[:, 4 * blk : 4 * blk + 4, :, :],
                        in_=vt_d[:, 4 * blk : 4 * blk + 4, :, :],
                    )

                # startup-critical DMA order: pair-0 K/Q weights + first K/Q
                # columns, then V / remaining K/Q interleaved
                prefetch_wk(0)
                prefetch_wq(0)
                dma_ktx(0)
                dma_qtx(0)
                dma_qtx(1)
                dma_wv(True)
                dma_vt(0)
                prefetch_wk(1)
                prefetch_wq(1)
                dma_ktx(1)
                dma_vt(1)
                dma_ktx(2)
                dma_vt(2)
                dma_ktx(3)
                dma_qtx(2)
                dma_vt(3)
                dma_qtx(3)
                dma_wv(False)

                # ---------- projection "filler" groups ----------
                def k_group(p, g):
                    pj = pjsum.tile([128, 512], F32, tag="pj")
                    for mo in range(N_MO):
                        nc.tensor.matmul(
                            pj[:],
                            cur_wk[p][:, mo, :],
                            ktx[:, mo, g * 512 : (g + 1) * 512],
                            start=(mo == 0),
                            stop=(mo == N_MO - 1),
                        )
                    nc.vector.tensor_copy(kt_sb[p][:, g * 512 : (g + 1) * 512], pj[:])

                def q_group(p, g, bank=None):
                    if bank is None:
                        pj = pjsum.tile([128, 512], F32, tag="pj")
                    else:
                        pj = spsum.tile([128, QW], F32, tag=bank, name="qpj")[:, 0:512]
                    for mo in range(N_MO):
                        nc.tensor.matmul(
                            pj[:],
                            cur_wq[p][:, mo, :],
                            qtx[:, mo * S + g * 512 : mo * S + (g + 1) * 512],
                            start=(mo == 0),
                            stop=(mo == N_MO - 1),
                        )
                    nc.vector.tensor_copy(qt_sb[p][:, g * 512 : (g + 1) * 512], pj[:])

                def v_group(sc, hp):
                    # one pair's two head-slots (128 wv cols), chunk sc
                    pj = pjsum.tile([128, 512], F32, tag="pj")
                    for mo in range(N_MO):
                        nc.tensor.matmul(
                            pj[:, 0:128],
                            vtx[:, sc, mo, :],
                            wv_sb[:, mo, hp * 128 : (hp + 1) * 128],
                            start=(mo == 0),
                            stop=(mo == N_MO - 1),
                        )
                    nc.vector.tensor_copy(
                        v_all[:, sc, 2 * hp : 2 * hp + 2, 1:65],
                        pj[:, 0:128].rearrange("p (h w) -> p h w", h=2),
                    )

                # out-projection:
                #  - g1 chunks: partial chains over pairs 0-2 staged to SBUF
                #    f32 (dead qtx space) inside units 6-7; the drain tail
                #    re-feeds each staged partial into PSUM with an identity
                #    matmul (float32r moving operand: full rate, ~tf32
                #    rounding) on top of the pair-3 matmul, then DMAs the
                #    PSUM result out directly.
                #  - g0 chunks: full 4-pair chains inside unit 7.
                def stage(qc, dmc):
                    pj = pjsum.tile([128, 512], F32, tag="pj", name="st")
                    for p2 in range(NPC - 1):
                        nc.tensor.matmul(
                            pj[:],
                            heads_chunk(p2, qc),
                            wo_sb[:, p2, dmc * 512 : (dmc + 1) * 512],
                            start=(p2 == 0),
                            stop=(p2 == NPC - 2),
                        )
                    nc.vector.tensor_copy(stage_slot((qc - 8) * 2 + dmc), pj[:])

                def full_chain(qc):
                    # both dm-halves of one q-chunk -> one bf16 store
                    fo = fout.tile([128, 1024], BF16, tag="fo")
                    for dmc in range(2):
                        pj = pjsum.tile([128, 512], F32, tag="pj", name="fc")
                        for p2 in range(NPC):
                            nc.tensor.matmul(
                                pj[:],
                                heads_chunk(p2, qc),
                                wo_sb[:, p2, dmc * 512 : (dmc + 1) * 512],
                                start=(p2 == 0),
                                stop=(p2 == NPC - 1),
                            )
                        nc.vector.tensor_copy(
                            fo[:, dmc * 512 : (dmc + 1) * 512], pj[:]
                        )
                    nc.sync.dma_start(out=out[qc * 128 : (qc + 1) * 128, :], in_=fo[:])

                # K/Q for unit (0,0) before attention starts; the three
                # chains use three different PSUM banks so they pipeline on
                # the PE instead of serializing on pj-bank WAR turnarounds
                k_group(0, 0)
                q_group(0, 0, bank="sp0")
                q_group(0, 1, bank="sp1")

                if dbg:
                    nc.sync.dma_start(out=d_kt[:], in_=kt_sb[0][:])
                    nc.sync.dma_start(out=d_qt[:], in_=qt_sb[0][:])

                def spread(items, nsteps=N_SC):
                    outl = [[] for _ in range(nsteps)]
                    for i, it in enumerate(items):
                        outl[(i * nsteps) // len(items)].append(it)
                    return outl

                def K(p, g):
                    return lambda: k_group(p, g)

                def Q(p, g):
                    return lambda: q_group(p, g)

                def V(sc, hp):
                    return lambda: v_group(sc, hp)

                def ST(qc, dmc):
                    return lambda: stage(qc, dmc)

                def FC(qc):
                    return lambda: full_chain(qc)

                def PF(p):
                    return [lambda p=p: prefetch_wk(p), lambda p=p: prefetch_wq(p)]

                # per-unit filler schedules, keyed by unit index in
                # U_ORDER = (0,g0),(1,g0),(0,g1),(1,g1),(2,g0),(3,g0),
                # (2,g1),(3,g1). Deadlines: v(sc,hp) by pair hp's first
                # unit, step sc+2; k(p,g) by pair p's first unit, step 4g;
                # q(p,gh) by the start of unit (p,gh). Entries are ordered
                # by staging-DMA arrival so a stalled group never blocks
                # the single projection-PSUM bank for a ready one.
                sched = {}
                sched[0] = [[] for _ in range(N_SC)]
                for step, fills in {
                    0: [V(0, 0), V(1, 0)],
                    1: [V(2, 0)],
                    2: [V(3, 0)],
                    3: [K(0, 1)],
                    4: [V(4, 0)],
                    5: [V(5, 0), K(1, 0)],
                    6: [V(6, 0)],
                    7: [V(7, 0), K(0, 2)],
                    8: [V(8, 0), Q(1, 0)],
                    9: [V(9, 0)],
                    10: [V(10, 0), K(0, 3)],
                    11: [V(11, 0), Q(1, 1)],
                    12: [V(12, 0)],
                    13: [V(13, 0)],
                    14: [V(14, 0), V(0, 1)],
                    15: [V(15, 0), V(1, 1)],
                }.items():
                    sched[0][step] = fills
                sched[1] = [[] for _ in range(N_SC)]
                for step, fills in {
                    0: [V(2, 1)],
                    1: [V(3, 1), K(1, 1)],
                    2: [V(4, 1)],
                    3: [V(5, 1)],
                    4: [V(6, 1), K(1, 2)],
                    5: [V(7, 1)],
                    6: [V(8, 1)],
                    7: [V(9, 1), K(1, 3)],
                    8: [V(10, 1)] + PF(2),
                    9: [V(11, 1)],
                    10: [V(12, 1), Q(0, 2)],
                    11: [V(13, 1)],
                    12: [V(14, 1), Q(0, 3)],
                    13: [V(15, 1)],
                }.items():
                    sched[1][step] = fills
                sched[2] = spread([Q(1, 2), Q(1, 3), K(2, 0), K(2, 1)])
                sched[3] = spread(
                    PF(3)
                    + [V(sc, 2) for sc in range(10)]
                    + [K(2, 2), K(2, 3)]
                    + [Q(2, 0), Q(2, 1)]
                )
                sched[4] = [[] for _ in range(N_SC)]
                for step, fills in {
                    0: [K(3, 0)],
                    1: [V(0, 3)],
                    2: [K(3, 1)],
                    3: [V(1, 3)],
                    4: [K(3, 2)],
                    5: [V(2, 3)],
                    6: [K(3, 3)],
                    7: [Q(3, 0), V(3, 3)],
                    8: [V(10, 2), V(4, 3)],
                    9: [Q(3, 1), V(11, 2)],
                    10: [V(12, 2), V(5, 3)],
                    11: [V(13, 2)],
                    12: [V(14, 2)],
                    13: [V(15, 2)],
                }.items():
                    sched[4][step] = fills
                sched[5] = spread(
                    [V(sc, 3) for sc in range(6, N_SC)] + [Q(2, 2), Q(2, 3)]
                )
                sched[6] = spread(
                    [Q(3, 2), Q(3, 3)] + [FC(qc) for qc in range(6)]
                )
                sched[7] = spread(
                    [FC(6), FC(7)]
                    + [ST(qc, dmc) for qc in range(8, 16) for dmc in range(2)]
                )

                # o accumulator slot -> AP. 16 slots (h,qc) packed into PSUM
                # banks as 7+7+2 (bank-straddle constraint).
                def o_slot(tiles, h, qc):
                    s = h * 8 + qc
                    if s < 7:
                        return tiles[0][:, s, :]
                    if s < 14:
                        return tiles[1][:, s - 7, :]
                    return tiles[2][:, s - 14, :]

                def emit_o(p, sc, h, o_ps, e_tiles):
                    e_sb = e_tiles.pop((sc, h))
                    for qc in range(8):
                        s = h * 8 + qc
                        nc.tensor.matmul(
                            o_slot(o_ps, h, qc),
                            e_sb[:, qc * 128 : (qc + 1) * 128],
                            v_all[:, sc, 2 * p + h, :],
                            start=(sc == 0 and s in (0, 7, 14)),
                            stop=(sc == N_SC - 1),
                            skip_group_check=True,
                        )

                # ---------------- attention (phase 2) ----------------
                U_ORDER = [(0, 0), (1, 0), (0, 1), (1, 1), (2, 0), (3, 0), (2, 1), (3, 1)]
                for u in range(2 * NPC if 2 in phases else 0):
                    p, g = U_ORDER[u]
                    if dbg and u == 2:
                        nc.sync.dma_start(out=d_onorm[:], in_=qt_sb[0][:, 0:QW])
                    if dbg and u == 6:
                        nc.sync.dma_start(
                            out=d_vall[:],
                            in_=v_all[:].rearrange("p a b c -> p (a b c)"),
                        )
                    if u == 6:
                        # V work done; release its staging space and load W_O
                        ves.close()
                        nc.sync.dma_start(out=wo_sb[:], in_=w_o[:])
                    o_ps = [
                        opsum.tile([128, 7, 65], F32, tag="oA", name="oA"),
                        opsum.tile([128, 7, 65], F32, tag="oB", name="oB"),
                        opsum.tile([128, 2, 65], F32, tag="oC", name="oC"),
                    ]
                    e_tiles = {}
                    for sc in range(N_SC):
                        for h in range(2):
                            lo, hi = h * 64, h * 64 + 64
                            sp = spsum.tile([128, QW], F32, tag=f"sp{h}")
                            for qc2 in range(QW // 512):
                                nc.tensor.matmul(
                                    sp[:, qc2 * 512 : (qc2 + 1) * 512],
                                    kt_sb[p][lo:hi, sc * 128 : (sc + 1) * 128],
                                    qt_sb[p][
                                        lo:hi,
                                        g * QW + qc2 * 512 : g * QW + (qc2 + 1) * 512,
                                    ],
                                    start=True,
                                    stop=True,
                                    skip_group_check=True,
                                )
                            e_sb = epool.tile([128, QW], BF16, tag=f"e{h}")
                            nc.scalar.activation(
                                e_sb[:],
                                sp[:],
                                mybir.ActivationFunctionType.Exp,
                                scale=0.125,
                            )
                            e_tiles[(sc, h)] = e_sb
                        # value matmuls lag two s-chunks behind the scores
                        if sc > 1:
                            for h in range(2):
                                emit_o(p, sc - 2, h, o_ps, e_tiles)
                        for fill in sched[u][sc]:
                            fill()
                    for h in range(2):
                        emit_o(p, N_SC - 2, h, o_ps, e_tiles)
                    emit_o(p, N_SC - 1, 0, o_ps, e_tiles)
                    # drain PSUM -> SBUF (bf16) + f32 denominators; oA only
                    # holds h=0 slots so it can drain while h=1 value
                    # matmuls still run
                    o_sb = npool.tile([128, 16, 65], BF16, tag="osb")
                    den = npool.tile([128, 16], F32, tag="den")
                    nc.vector.tensor_copy(o_sb[:, 0:7, :], o_ps[0][:])
                    nc.vector.tensor_copy(den[:, 0:7, None], o_ps[0][:, :, 0:1])
                    emit_o(p, N_SC - 1, 1, o_ps, e_tiles)
                    nc.vector.tensor_copy(o_sb[:, 7:14, :], o_ps[1][:])
                    nc.vector.tensor_copy(o_sb[:, 14:16, :], o_ps[2][:])
                    nc.vector.tensor_copy(den[:, 7:14, None], o_ps[1][:, :, 0:1])
                    nc.vector.tensor_copy(den[:, 14:16, None], o_ps[2][:, :, 0:1])
                    rec = npool.tile([128, 16], F32, tag="rec")
                    nc.vector.reciprocal_approx_fast(rec[:], den[:])
                    # normalized flipped heads, qc-major, into the dead qt
                    # half: col = qc*128 + h*64 + v
                    o_norm = qt_sb[p][:, g * QW : (g + 1) * QW].rearrange(
                        "p (a b c) -> p b a c", a=8, b=2, c=64
                    )
                    nc.vector.tensor_mul(
                        o_norm,
                        o_sb[:, :, 1:65].rearrange("p (h q) c -> p h q c", h=2),
                        rec[:].rearrange("p (h q) -> p h q", h=2)[
                            :, :, :, None
                        ].broadcast_to([128, 2, 8, 64]),
                    )
                    # xbar transposes -> pair-stacked headsT, emitted only
                    # once the destination is dead: pairs 0-1 overwrite
                    # their kt tile after their second (g1) unit; pairs 2-3
                    # write ktx right at their own unit's end (all K
                    # projection groups have been emitted by then)
                    if p < 2 and g == 1:
                        for g2 in range(2):
                            nc.sync.dma_start_transpose(
                                out=heads_dst(p, g2),
                                in_=qt_sb[p][:, g2 * QW : (g2 + 1) * QW],
                            )
                    elif p >= 2:
                        nc.sync.dma_start_transpose(
                            out=heads_dst(p, g),
                            in_=qt_sb[p][:, g * QW : (g + 1) * QW],
                        )

                if dbg:
                    nc.sync.dma_start(out=d_heads[:], in_=kt_sb[0][:])

                # drain unused fillers (for phases subsets)
                if 2 not in phases:
                    ves.close()
                    nc.sync.dma_start(out=wo_sb[:], in_=w_o[:])
                    for u in range(2 * NPC):
                        for step in sched.get(u, []):
                            for fill in step:
                                fill()

            # ---------------- g1 out-projection tails ----------------
            # pair-3 matmul plus an identity(float32r) re-feed of the staged
            # pairs-0-2 partial into the same PSUM accumulation, then DMA
            # the finished chunk straight from PSUM.
            with (
                tc.tile_pool(name="psum_t", bufs=6, space="PSUM") as tpsum,
                tc.tile_pool(name="fout2", bufs=4) as fout2,
            ):
                if 3 in phases:
                    def tail2(qc):
                        fo = fout2.tile([128, 1024], BF16, tag="fo")
                        for dmc in range(2):
                            tl = tpsum.tile([128, 512], F32, tag="tl")
                            # staged-partial re-feed first: it has no
                            # dependency on the final transpose, so these
                            # matmuls keep the PE busy (and warm) during the
                            # drain/normalize/transpose window
                            nc.tensor.matmul(
                                tl[:],
                                ident_bf[:],
                                stage_slot((qc - 8) * 2 + dmc),
                                start=True,
                                stop=False,
                            )
                            nc.tensor.matmul(
                                tl[:],
                                heads_chunk(NPC - 1, qc),
                                wo_sb[:, NPC - 1, dmc * 512 : (dmc + 1) * 512],
                                start=False,
                                stop=True,
                            )
                            if dmc == 0:
                                nc.scalar.copy(fo[:, 0:512], tl[:])
                            else:
                                nc.vector.tensor_copy(fo[:, 512:1024], tl[:])
                        nc.sync.dma_start(
                            out=out[qc * 128 : (qc + 1) * 128, :], in_=fo[:]
                        )

                    for qc in range(8, 16):
                        tail2(qc)
    nc.compile()
    return nc


_NC_CACHE = {}


def _get_nc():
    if "nc" not in _NC_CACHE:
        _NC_CACHE["nc"] = build()
    return _NC_CACHE["nc"]


def _prep_w3p(w):
    # [H, DM, D] -> pair-major [8 pairs, mi=128, mo=8, 128], bf16
    w3 = w.transpose(1, 0, 2).reshape(N_MO, 128, H * D).transpose(1, 0, 2)
    return np.ascontiguousarray(
        w3.reshape(128, N_MO, H // 2, 128).transpose(2, 0, 1, 3)
    ).astype(ml_dtypes.bfloat16)


def _prep_wv(w):
    # [H, DM, D] -> [mi=128, mo=8, (h d)=1024], bf16
    return np.ascontiguousarray(
        w.transpose(1, 0, 2).reshape(N_MO, 128, H * D).transpose(1, 0, 2)
    ).astype(ml_dtypes.bfloat16)


def _prep_wo(w):
    # [H*D=1024, DM] -> [mi=128, chunk=8, DM], bf16
    return np.ascontiguousarray(w.reshape(8, 128, DM).transpose(1, 0, 2)).astype(
        ml_dtypes.bfloat16
    )


def _prep_xt(x):
    # [S, DM] -> transposed [128(dm within mo), mo=8, S], bf16
    return np.ascontiguousarray(
        x.T.reshape(N_MO, 128, x.shape[0]).transpose(1, 0, 2)
    ).astype(ml_dtypes.bfloat16)


def _prep_vt(x):
    # [S, DM] -> [128(dm within mo), sc=16, mo=8, 128(s within chunk)]
    return np.ascontiguousarray(
        x.reshape(N_SC, 128, N_MO, 128).transpose(3, 0, 2, 1)
    ).astype(ml_dtypes.bfloat16)


def kernel(Q, K, V, W_Q, W_K, W_V, W_O, _trace=False):
    Q = np.asarray(Q, dtype=np.float32)
    K = np.asarray(K, dtype=np.float32)
    V = np.asarray(V, dtype=np.float32)
    wq8 = _prep_w3p(np.asarray(W_Q, dtype=np.float32))  # [8 pairs, ...]
    wk8 = _prep_w3p(np.asarray(W_K, dtype=np.float32))
    wv8 = _prep_wv(np.asarray(W_V, dtype=np.float32))  # [128, 8, 1024]
    wo8 = _prep_wo(np.asarray(W_O, dtype=np.float32))  # [128, 8, DM]

    qt_b = [_prep_xt(Q[b]) for b in range(B)]
    kt_b = [_prep_xt(K[b]) for b in range(B)]
    vt_b = [_prep_vt(V[b]) for b in range(B)]
    wq_h = [np.ascontiguousarray(wq8[hh * NPC : (hh + 1) * NPC]) for hh in range(2)]
    wk_h = [np.ascontiguousarray(wk8[hh * NPC : (hh + 1) * NPC]) for hh in range(2)]
    wv_h = [
        np.ascontiguousarray(wv8[:, :, hh * 512 : (hh + 1) * 512]) for hh in range(2)
    ]
    wo_h = [
        np.ascontiguousarray(wo8[:, hh * NPC : (hh + 1) * NPC, :]) for hh in range(2)
    ]

    in_maps = []
    for c in range(N_CORES):
        b, hh = c // 2, c % 2
        in_maps.append(
            {
                "QT": qt_b[b],
                "KT": kt_b[b],
                "VTs": vt_b[b],
                "WQP": wq_h[hh],
                "WKP": wk_h[hh],
                "WV3": wv_h[hh],
                "WO3": wo_h[hh],
            }
        )

    nc = _get_nc()
    res = run_bass_kernel_spmd(nc, in_maps, list(range(N_CORES)), trace=_trace)
    out = np.empty((B, S, DM), dtype=np.float32)
    for b in range(B):
        out[b] = np.asarray(res.results[2 * b]["out"], dtype=np.float32) + np.asarray(
            res.results[2 * b + 1]["out"], dtype=np.float32
        )
    if _trace:
        kernel._last_results = res
    return out


# revision 18
# speedup vs baseline: 1.0288x; 1.0161x over previous
"""Multi-head attention (B=4, S=2048, H=16, d_model=1024, d_k=d_v=64) on 8
Trainium2 NeuronCores.

Sharding (v2): 8 cores = 4 batches x 2 head-halves (tensor-parallel over
heads, per the W_Q/W_K/W_V head-split + W_O row-split scheme). Each core
computes 8 heads (4 pairs) over the FULL query range S=2048 for its batch,
projects K/V only for its own heads (no duplicated projection work), runs
its partial output projection against its W_O row block, and the host sums
the two partial outputs per batch (the all-reduce).

Host prep: Q/K/V transposed to [d_model, seq] (V blocked by s-chunk) and
cast to bf16, as are all weights; W_Q/W_K pair-major; W_V/W_O sliced per
head-half.

Per-core pipeline: 8 attention units (pair p, query-half g), pair-major
order. Per unit, the baseline-style software pipeline: scoresT chunk =
kt.T @ qt into PSUM, e = exp(s/8) on ACT (the pacing engine), flipped
value matmul o[q, 65] = e-chunk.T @ [1|v] accumulated over s-chunks with
column 0 the softmax denominator. K/Q/V projection groups and the
output-projection partials run as PE filler inside the ACT-paced loop.

Normalized heads are written qc-major into the unit's dead qt half, then a
single DMA xbar transpose per unit produces pair-stacked headsT in the dead
ktx staging area (no PE transposes). Output projection: partial chains over
pairs 0-2 are staged to SBUF f32 (dead qtx space) as filler; pair-3 tail
matmul + DVE/GPSIMD add completes each chunk (g0 chunks inside unit 7,
g1 chunks in the drain tail).

PSUM note: start_tensor_calc marks the whole 2KB bank pending-zero, so
interleaved per-slot accumulation groups sharing a bank must issue exactly
one start (first slot); the other slots' first writes land on pending-zero
bytes, which the hardware treats as overwrite.
"""

import contextlib
import os
import sys

for _p in ("/opt/trn_rl_repo", "/root/.axon_site/_ro/trn_rl_repo"):
    if os.path.isdir(_p) and _p not in sys.path:
        sys.path.insert(0, _p)

import numpy as np
import ml_dtypes

import concourse.bass as bass  # noqa: F401
import concourse.tile as tile
from concourse import bacc, mybir
from concourse.bass_utils import run_bass_kernel_spmd
from concourse.masks import make_identity

F32 = mybir.dt.float32
F32R = mybir.dt.float32r
BF16 = mybir.dt.bfloat16

B, S, DM = 4, 2048, 1024
H, D = 16, 64
N_CORES = 8
NPC = 4  # head pairs per core (8 heads)
N_SC = S // 128  # kv 128-chunks
N_MO = DM // 128  # model-dim 128-chunks
N_G = 2  # query halves per core
QW = 1024  # query width per attention unit


def build(n_cores=N_CORES, phases=(1, 2, 3), dbg=False):
    nc = bacc.Bacc("TRN2", target_bir_lowering=False, debug=False, num_devices=n_cores)

    # host-transposed activations, bf16 (full batch; core's own head slice
    # of the weights)
    qt_d = nc.dram_tensor("QT", [128, N_MO, S], BF16, kind="ExternalInput").ap()
    kt_d = nc.dram_tensor("KT", [128, N_MO, S], BF16, kind="ExternalInput").ap()
    vt_d = nc.dram_tensor(
        "VTs", [128, N_SC, N_MO, 128], BF16, kind="ExternalInput"
    ).ap()
    # pair-major: [pair, mi=128, mo=8, 128]
    w_q = nc.dram_tensor("WQP", [NPC, 128, N_MO, 128], BF16, kind="ExternalInput").ap()
    w_k = nc.dram_tensor("WKP", [NPC, 128, N_MO, 128], BF16, kind="ExternalInput").ap()
    w_v = nc.dram_tensor("WV3", [128, N_MO, 8 * D], BF16, kind="ExternalInput").ap()
    # [mi=128, pair=4, dm=1024]
    w_o = nc.dram_tensor("WO3", [128, NPC, DM], BF16, kind="ExternalInput").ap()
    out = nc.dram_tensor("out", [S, DM], BF16, kind="ExternalOutput").ap()
    if dbg:
        d_kt = nc.dram_tensor("d_kt", [128, S], BF16, kind="ExternalOutput").ap()
        d_qt = nc.dram_tensor("d_qt", [128, S], BF16, kind="ExternalOutput").ap()
        d_vall = nc.dram_tensor(
            "d_vall", [128, N_SC * 8 * 65], BF16, kind="ExternalOutput"
        ).ap()
        d_onorm = nc.dram_tensor("d_onorm", [128, QW], BF16, kind="ExternalOutput").ap()
        d_heads = nc.dram_tensor("d_heads", [128, S], BF16, kind="ExternalOutput").ap()

    with tile.TileContext(nc) as tc:
        with (
            tc.tile_pool(name="pers", bufs=1) as pers,
            tc.tile_pool(name="wkq", bufs=2) as wkq,
            tc.tile_pool(name="wop", bufs=1) as wop,
        ):
            wo_sb = wop.tile([128, NPC, DM], BF16, tag="wo", name="wo")
            ident_f32 = wop.tile([128, 128], F32, tag="identf", name="ident_f32")
            make_identity(nc, ident_f32[:])
            ident_bf = wop.tile([128, 128], BF16, tag="ident", name="ident_bf")
            nc.vector.tensor_copy(ident_bf[:], ident_f32[:])
            # v resident: per s-chunk block of 8 head-slots [1|v] (65 wide)
            v_all = pers.tile([128, N_SC, 8, 65], BF16, tag="v_all")
            nc.vector.memset(v_all[:, :, :, 0:1], 1.0)
            # kt[p]: pair-stacked [2*64, S]; qt[p]: [128, S], whose g-halves
            # are later reused for normalized flipped heads (qc-major)
            kt_sb = [
                pers.tile([128, S], BF16, tag=f"kt{p}", name=f"kt{p}")
                for p in range(NPC)
            ]
            qt_sb = [
                pers.tile([128, S], BF16, tag=f"qt{p}", name=f"qt{p}")
                for p in range(NPC)
            ]
            # resident staged transposed inputs for projections; ktx rows
            # double as headsT homes (mo-slot p holds pair p's headsT) once
            # the K projections have consumed them
            ktx = pers.tile([128, N_MO, S], BF16, tag="ktx")
            qtx = pers.tile([128, N_MO * S], BF16, tag="qtx")

            def heads_home(p):
                # pairs 0-1: own kt tile (dead after their second unit);
                # pairs 2-3: ktx slots (K staging is fully consumed by the
                # time their transposes run)
                return kt_sb[p] if p < 2 else ktx[:, p, :]

            def heads_dst(p, g):
                # [128, 8, 128] chunked-transpose target
                return heads_home(p)[:, g * QW : (g + 1) * QW].rearrange(
                    "p (c q) -> p c q", q=128
                )

            def heads_chunk(p, qc):
                # out-projection stationary: [128 hv, 128 q] for global qc
                return heads_home(p)[:, qc * 128 : (qc + 1) * 128]

            def stage_slot(i):
                # 16 bf16 staging slots in dead qtx space (bf16 so the tail
                # identity-matmul re-feed stays a plain bf16 matmul)
                return qtx[:, i * 512 : (i + 1) * 512]

            with (
                tc.tile_pool(name="psum_sp", bufs=1, space="PSUM") as spsum,
                tc.tile_pool(name="psum_o", bufs=1, space="PSUM") as opsum,
                tc.tile_pool(name="psum_pj", bufs=1, space="PSUM") as pjsum,
                tc.tile_pool(name="epool", bufs=4) as epool,
                tc.tile_pool(name="npool", bufs=1) as npool,
                tc.tile_pool(name="fout", bufs=2) as fout,
            ):
                ves = contextlib.ExitStack()
                vpool = ves.enter_context(tc.tile_pool(name="vpool", bufs=1))
                vtx = vpool.tile([128, N_SC, N_MO, 128], BF16, tag="vtx")
                wv_sb = vpool.tile([128, N_MO, 8 * D], BF16, tag="wv")

                cur_wk, cur_wq = {}, {}

                def prefetch_wk(p):
                    wkt = wkq.tile([128, N_MO, 128], BF16, tag="wk", name="wk")
                    cur_wk[p] = wkt
                    nc.sync.dma_start(out=wkt[:], in_=w_k[p])

                def prefetch_wq(p):
                    wqt = wkq.tile([128, N_MO, 128], BF16, tag="wq", name="wq")
                    cur_wq[p] = wqt
                    nc.sync.dma_start(out=wqt[:], in_=w_q[p])

                def dma_ktx(g, half=None):
                    mo0, mo1 = (0, N_MO) if half is None else (4 * half, 4 * half + 4)
                    nc.sync.dma_start(
                        out=ktx[:, mo0:mo1, g * 512 : (g + 1) * 512],
                        in_=kt_d[:, mo0:mo1, g * 512 : (g + 1) * 512],
                    )

                def dma_qtx(g, half=None):
                    mo0, mo1 = (0, N_MO) if half is None else (4 * half, 4 * half + 4)
                    nc.sync.dma_start(
                        out=qtx[:].rearrange("p (m s) -> p m s", s=S)[
                            :, mo0:mo1, g * 512 : (g + 1) * 512
                        ],
                        in_=qt_d[:, mo0:mo1, g * 512 : (g + 1) * 512],
                    )

                def dma_wv(first):
                    if first:
                        nc.sync.dma_start(
                            out=wv_sb[:, :, 0:256], in_=w_v[:, :, 0:256]
                        )
                    else:
                        nc.sync.dma_start(
                            out=wv_sb[:, :, 256:512], in_=w_v[:, :, 256:512]
                        )

                def dma_vt(blk):
                    nc.sync.dma_start(
                        out=vtx[:, 4 * blk : 4 * blk + 4, :, :],
                        in_=vt_d[:, 4 * blk : 4 * blk + 4, :, :],
                    )

                # startup-critical DMA order: pair-0 K/Q weights + first K/Q
                # columns, then V / remaining K/Q interleaved
                prefetch_wk(0)
                prefetch_wq(0)
                dma_ktx(0, 0)
                dma_ktx(0, 1)
                dma_qtx(0, 0)
                dma_qtx(0, 1)
                dma_qtx(1, 0)
                dma_qtx(1, 1)
                dma_wv(True)
                dma_vt(0)
                prefetch_wk(1)
                prefetch_wq(1)
                dma_ktx(1)
                dma_vt(1)
                dma_ktx(2)
                dma_vt(2)
                dma_ktx(3)
                dma_qtx(2)
                dma_vt(3)
                dma_qtx(3)
                dma_wv(False)

                # ---------- projection "filler" groups ----------
                def k_group(p, g):
                    pj = pjsum.tile([128, 512], F32, tag="pj")
                    for mo in range(N_MO):
                        nc.tensor.matmul(
                            pj[:],
                            cur_wk[p][:, mo, :],
                            ktx[:, mo, g * 512 : (g + 1) * 512],
                            start=(mo == 0),
                            stop=(mo == N_MO - 1),
                        )
                    nc.vector.tensor_copy(kt_sb[p][:, g * 512 : (g + 1) * 512], pj[:])

                def q_group(p, g, bank=None):
                    if bank is None:
                        pj = pjsum.tile([128, 512], F32, tag="pj")
                    else:
                        pj = spsum.tile([128, QW], F32, tag=bank, name="qpj")[:, 0:512]
                    for mo in range(N_MO):
                        nc.tensor.matmul(
                            pj[:],
                            cur_wq[p][:, mo, :],
                            qtx[:, mo * S + g * 512 : mo * S + (g + 1) * 512],
                            start=(mo == 0),
                            stop=(mo == N_MO - 1),
                        )
                    nc.vector.tensor_copy(qt_sb[p][:, g * 512 : (g + 1) * 512], pj[:])

                def v_group(sc, hp):
                    # one pair's two head-slots (128 wv cols), chunk sc
                    pj = pjsum.tile([128, 512], F32, tag="pj")
                    for mo in range(N_MO):
                        nc.tensor.matmul(
                            pj[:, 0:128],
                            vtx[:, sc, mo, :],
                            wv_sb[:, mo, hp * 128 : (hp + 1) * 128],
                            start=(mo == 0),
                            stop=(mo == N_MO - 1),
                        )
                    nc.vector.tensor_copy(
                        v_all[:, sc, 2 * hp : 2 * hp + 2, 1:65],
                        pj[:, 0:128].rearrange("p (h w) -> p h w", h=2),
                    )

                # out-projection:
                #  - g1 chunks: partial chains over pairs 0-2 staged to SBUF
                #    f32 (dead qtx space) inside units 6-7; the drain tail
                #    re-feeds each staged partial into PSUM with an identity
                #    matmul (float32r moving operand: full rate, ~tf32
                #    rounding) on top of the pair-3 matmul, then DMAs the
                #    PSUM result out directly.
                #  - g0 chunks: full 4-pair chains inside unit 7.
                def stage(qc, dmc):
                    pj = pjsum.tile([128, 512], F32, tag="pj", name="st")
                    for p2 in range(NPC - 1):
                        nc.tensor.matmul(
                            pj[:],
                            heads_chunk(p2, qc),
                            wo_sb[:, p2, dmc * 512 : (dmc + 1) * 512],
                            start=(p2 == 0),
                            stop=(p2 == NPC - 2),
                        )
                    nc.vector.tensor_copy(stage_slot((qc - 8) * 2 + dmc), pj[:])

                def full_chain(qc):
                    # both dm-halves of one q-chunk -> one bf16 store
                    fo = fout.tile([128, 1024], BF16, tag="fo")
                    for dmc in range(2):
                        pj = pjsum.tile([128, 512], F32, tag="pj", name="fc")
                        for p2 in range(NPC):
                            nc.tensor.matmul(
                                pj[:],
                                heads_chunk(p2, qc),
                                wo_sb[:, p2, dmc * 512 : (dmc + 1) * 512],
                                start=(p2 == 0),
                                stop=(p2 == NPC - 1),
                            )
                        nc.vector.tensor_copy(
                            fo[:, dmc * 512 : (dmc + 1) * 512], pj[:]
                        )
                    nc.sync.dma_start(out=out[qc * 128 : (qc + 1) * 128, :], in_=fo[:])

                # K/Q for unit (0,0) before attention starts
                k_group(0, 0)
                q_group(0, 0)
                q_group(0, 1)

                if dbg:
                    nc.sync.dma_start(out=d_kt[:], in_=kt_sb[0][:])
                    nc.sync.dma_start(out=d_qt[:], in_=qt_sb[0][:])

                def spread(items, nsteps=N_SC):
                    outl = [[] for _ in range(nsteps)]
                    for i, it in enumerate(items):
                        outl[(i * nsteps) // len(items)].append(it)
                    return outl

                def K(p, g):
                    return lambda: k_group(p, g)

                def Q(p, g):
                    return lambda: q_group(p, g)

                def V(sc, hp):
                    return lambda: v_group(sc, hp)

                def ST(qc, dmc):
                    return lambda: stage(qc, dmc)

                def FC(qc):
                    return lambda: full_chain(qc)

                def PF(p):
                    return [lambda p=p: prefetch_wk(p), lambda p=p: prefetch_wq(p)]

                # per-unit filler schedules, keyed by unit index in
                # U_ORDER = (0,g0),(1,g0),(0,g1),(1,g1),(2,g0),(3,g0),
                # (2,g1),(3,g1). Deadlines: v(sc,hp) by pair hp's first
                # unit, step sc+2; k(p,g) by pair p's first unit, step 4g;
                # q(p,gh) by the start of unit (p,gh). Entries are ordered
                # by staging-DMA arrival so a stalled group never blocks
                # the single projection-PSUM bank for a ready one.
                sched = {}
                sched[0] = [[] for _ in range(N_SC)]
                for step, fills in {
                    0: [V(0, 0), V(1, 0)],
                    1: [V(2, 0)],
                    2: [V(3, 0)],
                    3: [K(0, 1)],
                    4: [V(4, 0)],
                    5: [V(5, 0), K(1, 0)],
                    6: [V(6, 0)],
                    7: [V(7, 0), K(0, 2)],
                    8: [V(8, 0), Q(1, 0)],
                    9: [V(9, 0)],
                    10: [V(10, 0), K(0, 3)],
                    11: [V(11, 0), Q(1, 1)],
                    12: [V(12, 0)],
                    13: [V(13, 0)],
                    14: [V(14, 0), V(0, 1)],
                    15: [V(15, 0), V(1, 1)],
                }.items():
                    sched[0][step] = fills
                sched[1] = [[] for _ in range(N_SC)]
                for step, fills in {
                    0: [V(2, 1)],
                    1: [V(3, 1), K(1, 1)],
                    2: [V(4, 1)],
                    3: [V(5, 1)],
                    4: [V(6, 1), K(1, 2)],
                    5: [V(7, 1)],
                    6: [V(8, 1)],
                    7: [V(9, 1), K(1, 3)],
                    8: [V(10, 1)] + PF(2),
                    9: [V(11, 1)],
                    10: [V(12, 1), Q(0, 2)],
                    11: [V(13, 1)],
                    12: [V(14, 1), Q(0, 3)],
                    13: [V(15, 1)],
                }.items():
                    sched[1][step] = fills
                sched[2] = spread([Q(1, 2), Q(1, 3), K(2, 0), K(2, 1)])
                sched[3] = spread(
                    PF(3)
                    + [V(sc, 2) for sc in range(10)]
                    + [K(2, 2), K(2, 3)]
                    + [Q(2, 0), Q(2, 1)]
                )
                sched[4] = [[] for _ in range(N_SC)]
                for step, fills in {
                    0: [K(3, 0)],
                    1: [V(0, 3)],
                    2: [K(3, 1)],
                    3: [V(1, 3)],
                    4: [K(3, 2)],
                    5: [V(2, 3)],
                    6: [K(3, 3)],
                    7: [Q(3, 0), V(3, 3)],
                    8: [V(10, 2), V(4, 3)],
                    9: [Q(3, 1), V(11, 2)],
                    10: [V(12, 2), V(5, 3)],
                    11: [V(13, 2)],
                    12: [V(14, 2)],
                    13: [V(15, 2)],
                }.items():
                    sched[4][step] = fills
                sched[5] = spread(
                    [V(sc, 3) for sc in range(6, N_SC)] + [Q(2, 2), Q(2, 3)]
                )
                sched[6] = spread(
                    [Q(3, 2), Q(3, 3)] + [FC(qc) for qc in range(6)]
                )
                sched[7] = spread(
                    [FC(6), FC(7)]
                    + [ST(qc, dmc) for qc in range(8, 16) for dmc in range(2)]
                )

                # o accumulator slot -> AP. 16 slots (h,qc) packed into PSUM
                # banks as 7+7+2 (bank-straddle constraint).
                def o_slot(tiles, h, qc):
                    s = h * 8 + qc
                    if s < 7:
                        return tiles[0][:, s, :]
                    if s < 14:
                        return tiles[1][:, s - 7, :]
                    return tiles[2][:, s - 14, :]

                def emit_o(p, sc, h, o_ps, e_tiles):
                    e_sb = e_tiles.pop((sc, h))
                    for qc in range(8):
                        s = h * 8 + qc
                        nc.tensor.matmul(
                            o_slot(o_ps, h, qc),
                            e_sb[:, qc * 128 : (qc + 1) * 128],
                            v_all[:, sc, 2 * p + h, :],
                            start=(sc == 0 and s in (0, 7, 14)),
                            stop=(sc == N_SC - 1),
                            skip_group_check=True,
                        )

                # ---------------- attention (phase 2) ----------------
                U_ORDER = [(0, 0), (1, 0), (0, 1), (1, 1), (2, 0), (3, 0), (2, 1), (3, 1)]
                for u in range(2 * NPC if 2 in phases else 0):
                    p, g = U_ORDER[u]
                    if dbg and u == 2:
                        nc.sync.dma_start(out=d_onorm[:], in_=qt_sb[0][:, 0:QW])
                    if dbg and u == 6:
                        nc.sync.dma_start(
                            out=d_vall[:],
                            in_=v_all[:].rearrange("p a b c -> p (a b c)"),
                        )
                    if u == 6:
                        # V work done; release its staging space and load W_O
                        ves.close()
                        nc.sync.dma_start(out=wo_sb[:], in_=w_o[:])
                    o_ps = [
                        opsum.tile([128, 7, 65], F32, tag="oA", name="oA"),
                        opsum.tile([128, 7, 65], F32, tag="oB", name="oB"),
                        opsum.tile([128, 2, 65], F32, tag="oC", name="oC"),
                    ]
                    e_tiles = {}
                    for sc in range(N_SC):
                        for h in range(2):
                            lo, hi = h * 64, h * 64 + 64
                            sp = spsum.tile([128, QW], F32, tag=f"sp{h}")
                            for qc2 in range(QW // 512):
                                nc.tensor.matmul(
                                    sp[:, qc2 * 512 : (qc2 + 1) * 512],
                                    kt_sb[p][lo:hi, sc * 128 : (sc + 1) * 128],
                                    qt_sb[p][
                                        lo:hi,
                                        g * QW + qc2 * 512 : g * QW + (qc2 + 1) * 512,
                                    ],
                                    start=True,
                                    stop=True,
                                    skip_group_check=True,
                                )
                            e_sb = epool.tile([128, QW], BF16, tag=f"e{h}")
                            nc.scalar.activation(
                                e_sb[:],
                                sp[:],
                                mybir.ActivationFunctionType.Exp,
                                scale=0.125,
                            )
                            e_tiles[(sc, h)] = e_sb
                        # value matmuls lag two s-chunks behind the scores
                        if sc > 1:
                            for h in range(2):
                                emit_o(p, sc - 2, h, o_ps, e_tiles)
                        for fill in sched[u][sc]:
                            fill()
                    for h in range(2):
                        emit_o(p, N_SC - 2, h, o_ps, e_tiles)
                    emit_o(p, N_SC - 1, 0, o_ps, e_tiles)
                    # drain PSUM -> SBUF (bf16) + f32 denominators; oA only
                    # holds h=0 slots so it can drain while h=1 value
                    # matmuls still run
                    o_sb = npool.tile([128, 16, 65], BF16, tag="osb")
                    den = npool.tile([128, 16], F32, tag="den")
                    nc.vector.tensor_copy(o_sb[:, 0:7, :], o_ps[0][:])
                    nc.vector.tensor_copy(den[:, 0:7, None], o_ps[0][:, :, 0:1])
                    emit_o(p, N_SC - 1, 1, o_ps, e_tiles)
                    nc.vector.tensor_copy(o_sb[:, 7:14, :], o_ps[1][:])
                    nc.vector.tensor_copy(o_sb[:, 14:16, :], o_ps[2][:])
                    nc.vector.tensor_copy(den[:, 7:14, None], o_ps[1][:, :, 0:1])
                    nc.vector.tensor_copy(den[:, 14:16, None], o_ps[2][:, :, 0:1])
                    rec = npool.tile([128, 16], F32, tag="rec")
                    nc.vector.reciprocal_approx_fast(rec[:], den[:])
                    # normalized flipped heads, qc-major, into the dead qt
                    # half: col = qc*128 + h*64 + v
                    o_norm = qt_sb[p][:, g * QW : (g + 1) * QW].rearrange(
                        "p (a b c) -> p b a c", a=8, b=2, c=64
                    )
                    nc.vector.tensor_mul(
                        o_norm,
                        o_sb[:, :, 1:65].rearrange("p (h q) c -> p h q c", h=2),
                        rec[:].rearrange("p (h q) -> p h q", h=2)[
                            :, :, :, None
                        ].broadcast_to([128, 2, 8, 64]),
                    )
                    # xbar transposes -> pair-stacked headsT, emitted only
                    # once the destination is dead: pairs 0-1 overwrite
                    # their kt tile after their second (g1) unit; pairs 2-3
                    # write ktx right at their own unit's end (all K
                    # projection groups have been emitted by then)
                    if p < 2 and g == 1:
                        for g2 in range(2):
                            nc.sync.dma_start_transpose(
                                out=heads_dst(p, g2),
                                in_=qt_sb[p][:, g2 * QW : (g2 + 1) * QW],
                            )
                    elif p >= 2:
                        nc.sync.dma_start_transpose(
                            out=heads_dst(p, g),
                            in_=qt_sb[p][:, g * QW : (g + 1) * QW],
                        )

                if dbg:
                    nc.sync.dma_start(out=d_heads[:], in_=kt_sb[0][:])

                # drain unused fillers (for phases subsets)
                if 2 not in phases:
                    ves.close()
                    nc.sync.dma_start(out=wo_sb[:], in_=w_o[:])
                    for u in range(2 * NPC):
                        for step in sched.get(u, []):
                            for fill in step:
                                fill()

            # ---------------- g1 out-projection tails ----------------
            # pair-3 matmul plus an identity(float32r) re-feed of the staged
            # pairs-0-2 partial into the same PSUM accumulation, then DMA
            # the finished chunk straight from PSUM.
            with (
                tc.tile_pool(name="psum_t", bufs=6, space="PSUM") as tpsum,
                tc.tile_pool(name="fout2", bufs=4) as fout2,
            ):
                if 3 in phases:
                    def tail2(qc):
                        fo = fout2.tile([128, 1024], BF16, tag="fo")
                        for dmc in range(2):
                            tl = tpsum.tile([128, 512], F32, tag="tl")
                            # staged-partial re-feed first: it has no
                            # dependency on the final transpose, so these
                            # matmuls keep the PE busy (and warm) during the
                            # drain/normalize/transpose window
                            nc.tensor.matmul(
                                tl[:],
                                ident_bf[:],
                                stage_slot((qc - 8) * 2 + dmc),
                                start=True,
                                stop=False,
                            )
                            nc.tensor.matmul(
                                tl[:],
                                heads_chunk(NPC - 1, qc),
                                wo_sb[:, NPC - 1, dmc * 512 : (dmc + 1) * 512],
                                start=False,
                                stop=True,
                            )
                            if dmc == 0:
                                nc.scalar.copy(fo[:, 0:512], tl[:])
                            else:
                                nc.vector.tensor_copy(fo[:, 512:1024], tl[:])
                        nc.sync.dma_start(
                            out=out[qc * 128 : (qc + 1) * 128, :], in_=fo[:]
                        )

                    for qc in range(8, 16):
                        tail2(qc)
    nc.compile()
    return nc


_NC_CACHE = {}


def _get_nc():
    if "nc" not in _NC_CACHE:
        _NC_CACHE["nc"] = build()
    return _NC_CACHE["nc"]


def _prep_w3p(w):
    # [H, DM, D] -> pair-major [8 pairs, mi=128, mo=8, 128], bf16
    w3 = w.transpose(1, 0, 2).reshape(N_MO, 128, H * D).transpose(1, 0, 2)
    return np.ascontiguousarray(
        w3.reshape(128, N_MO, H // 2, 128).transpose(2, 0, 1, 3)
    ).astype(ml_dtypes.bfloat16)


def _prep_wv(w):
    # [H, DM, D] -> [mi=128, mo=8, (h d)=1024], bf16
    return np.ascontiguousarray(
        w.transpose(1, 0, 2).reshape(N_MO, 128, H * D).transpose(1, 0, 2)
    ).astype(ml_dtypes.bfloat16)


def _prep_wo(w):
    # [H*D=1024, DM] -> [mi=128, chunk=8, DM], bf16
    return np.ascontiguousarray(w.reshape(8, 128, DM).transpose(1, 0, 2)).astype(
        ml_dtypes.bfloat16
    )


def _prep_xt(x):
    # [S, DM] -> transposed [128(dm within mo), mo=8, S], bf16
    return np.ascontiguousarray(
        x.T.reshape(N_MO, 128, x.shape[0]).transpose(1, 0, 2)
    ).astype(ml_dtypes.bfloat16)


def _prep_vt(x):
    # [S, DM] -> [128(dm within mo), sc=16, mo=8, 128(s within chunk)]
    return np.ascontiguousarray(
        x.reshape(N_SC, 128, N_MO, 128).transpose(3, 0, 2, 1)
    ).astype(ml_dtypes.bfloat16)


def kernel(Q, K, V, W_Q, W_K, W_V, W_O, _trace=False):
    Q = np.asarray(Q, dtype=np.float32)
    K = np.asarray(K, dtype=np.float32)
    V = np.asarray(V, dtype=np.float32)
    wq8 = _prep_w3p(np.asarray(W_Q, dtype=np.float32))  # [8 pairs, ...]
    wk8 = _prep_w3p(np.asarray(W_K, dtype=np.float32))
    wv8 = _prep_wv(np.asarray(W_V, dtype=np.float32))  # [128, 8, 1024]
    wo8 = _prep_wo(np.asarray(W_O, dtype=np.float32))  # [128, 8, DM]

    qt_b = [_prep_xt(Q[b]) for b in range(B)]
    kt_b = [_prep_xt(K[b]) for b in range(B)]
    vt_b = [_prep_vt(V[b]) for b in range(B)]
    wq_h = [np.ascontiguousarray(wq8[hh * NPC : (hh + 1) * NPC]) for hh in range(2)]
    wk_h = [np.ascontiguousarray(wk8[hh * NPC : (hh + 1) * NPC]) for hh in range(2)]
    wv_h = [
        np.ascontiguousarray(wv8[:, :, hh * 512 : (hh + 1) * 512]) for hh in range(2)
    ]
    wo_h = [
        np.ascontiguousarray(wo8[:, hh * NPC : (hh + 1) * NPC, :]) for hh in range(2)
    ]

    in_maps = []
    for c in range(N_CORES):
        b, hh = c // 2, c % 2
        in_maps.append(
            {
                "QT": qt_b[b],
                "KT": kt_b[b],
                "VTs": vt_b[b],
                "WQP": wq_h[hh],
                "WKP": wk_h[hh],
                "WV3": wv_h[hh],
                "WO3": wo_h[hh],
            }
        )

    nc = _get_nc()
    res = run_bass_kernel_spmd(nc, in_maps, list(range(N_CORES)), trace=_trace)
    out = np.empty((B, S, DM), dtype=np.float32)
    for b in range(B):
        out[b] = np.asarray(res.results[2 * b]["out"], dtype=np.float32) + np.asarray(
            res.results[2 * b + 1]["out"], dtype=np.float32
        )
    if _trace:
        kernel._last_results = res
    return out
